# revision 44
# baseline (speedup 1.0000x reference)
"""AGNNConv on 8 Trainium2 NeuronCores (Bass/Tile).

Algorithm (matches reference up to fp noise):
    xn = x / ||x||;  w_e = beta * <xn[row_e], xn[col_e]>  (bounded in [-|b|,|b|])
    out[r] = (sum_e exp(w_e) * x[col_e] + exp(w_self_r) * x[r])
             / (sum_e exp(w_e) + exp(w_self_r))
Max-subtraction in the segment softmax is skipped (w is bounded), making the
softmax a pure segment-sum.

Distribution: each core owns destination rows [6250k, 6250(k+1)); edges are
assigned to their destination-row owner -> no cross-core communication.

Per-core dataflow (edges sorted by destination row, 128-edge tiles aligned to
128-row blocks):
  - x[col] fetched with gpsimd.dma_gather — the only per-edge DMA descriptors
    (the Q7 generates descriptors at ~10ns each, so row-side gather/scatter
    descriptors are eliminated entirely):
  - row-side "gather" of xn[row]: one-hot EQT (is_equal of a broadcast row-id
    stream against a partition iota) matmul-multiplied with the SBUF-resident
    normalized row-shard block (PE, bf16)
  - row-side "scatter" of [s*x_col | s]: one-hot EQ^T @ rhs (PE, bf16)
    accumulated in PSUM per row-block run, added into a pinned f32 SBUF
    accumulator
Epilogue adds the self-loop term and normalizes. bf16 is used only for the
exact 0/1 one-hot operands and the matmul value inputs; accumulation is f32.
"""

import math

import numpy as np

N = 50000
D = 64
CORES = 8
SH = N // CORES          # 6250 destination rows per core
NB = (SH + 127) // 128   # 49 row blocks (last block has 106 rows)
HALF = 25000             # column-half size (dma_gather idx is int16)
CTILE = 40               # tiles per gather chunk (chunk = CTILE*128 edges)
P = 128

_graph_cache = {}


def _bf16np():
    from concourse import mybir
    return mybir.dt.np(mybir.dt.bfloat16)


def _wrap16(a):
    """int16 idx array [C] -> [128, C//16] in dma_gather's wrapped-16 layout
    (idx i lives at [i % 16, i // 16]), replicated to the 8 q7 cores."""
    w = a.reshape(-1, 16).T
    return np.ascontiguousarray(np.tile(w, (8, 1)))


def _build(tiles_per_hb, beta_val):
    """tiles_per_hb: [2][NB] tile counts per (col-half, row-block) — identical
    across cores (host pads every core to these counts)."""
    from concourse import bacc, hw_specs, library_config, mybir, tile

    # The Q7 ucode generates gather/scatter descriptors at ~10ns each (the
    # stock constant models plain SWDGE dma_start at 0.34ns/desc). Without
    # this, the Tile scheduler thinks a 5120-edge dma_gather takes ~2.7us
    # (actual ~49us) and serializes compute after it instead of overlapping.
    hw_specs.TRN2Spec.SWDGE_NS_PER_DESCRIPTOR = 9.5

    f32 = mybir.dt.float32
    bf16 = mybir.dt.bfloat16
    i16 = mybir.dt.int16
    AX = mybir.AxisListType
    OP = mybir.AluOpType
    AF = mybir.ActivationFunctionType

    # flat tile list: (half, block) in stream order. h1 runs blocks in
    # DESCENDING order so blocks finalize progressively during the second
    # half and their epilogue pieces overlap the remaining gathers.
    tile_blocks = []
    tile_half = []
    for h in range(2):
        border = range(NB)
        for b in border:
            for _ in range(tiles_per_hb[h][b]):
                tile_blocks.append(b)
                tile_half.append(h)
    n_tiles = len(tile_blocks)
    n_chunks = (n_tiles + CTILE - 1) // CTILE
    C = CTILE * P

    nc = bacc.Bacc(None, target_bir_lowering=False, debug=False)
    x_ext = nc.declare_dram_parameter("x", [N, D], f32, isOutput=False)
    xs_ext = nc.declare_dram_parameter("xshard", [SH, D], f32, isOutput=False)
    gc_ext = nc.declare_dram_parameter("gc", [n_chunks, P, C // 16], i16,
                                       isOutput=False)
    rl_ext = nc.declare_dram_parameter("rloc", [n_chunks, P, CTILE], bf16,
                                       isOutput=False)
    rt_ext = nc.declare_dram_parameter("rlt", [n_chunks, C], bf16,
                                       isOutput=False)
    mk_ext = nc.declare_dram_parameter("mk", [n_chunks, P, CTILE], f32,
                                       isOutput=False)
    out_ext = nc.declare_dram_parameter("out", [SH, D], f32, isOutput=True)

    # dma_gather mis-handles nonzero in_ap base offsets: upper col half gets
    # its own zero-offset copy of x.
    xhi_t = nc.dram_tensor("x_hi", [HALF, D], f32)

    FULLB = SH // P * P  # 6144 rows in full blocks
    RUNT = SH - FULLB    # 106

    with tile.TileContext(nc) as tc:
        with tc.tile_pool(name="pin", bufs=1) as pin, \
             tc.tile_pool(name="io", bufs=4) as io, \
             tc.tile_pool(name="ps", bufs=2, space="PSUM") as ps, \
             tc.tile_pool(name="psacc", bufs=2, space="PSUM") as psacc:

            nc.gpsimd.load_library(library_config.mlp)

            def emit_xhi_copy():
                for j in range(4):
                    nc.scalar.dma_start(
                        out=xhi_t[j * 6250:(j + 1) * 6250, :],
                        in_=x_ext[HALF + j * 6250:HALF + (j + 1) * 6250, :])

            # --- pinned constants & state ---------------------------------
            bfnp = _bf16np()
            iotf_d = nc.inline_tensor(
                np.tile(np.arange(P, dtype=np.float32), (P, 1)).astype(bfnp),
                name="iotfc")
            iop_d = nc.inline_tensor(
                np.arange(P, dtype=np.float32).reshape(P, 1).astype(bfnp),
                name="iopc")
            iotf = pin.tile([P, P], bf16)
            nc.sync.dma_start(out=iotf[:], in_=iotf_d[:, :])
            iop = pin.tile([P, 1], bf16)
            nc.sync.dma_start(out=iop[:], in_=iop_d[:, :])
            # normalized row shard, block-major; runt tail zeroed
            xnp = pin.tile([P, NB, D], f32)
            xnpb = pin.tile([P, NB, D], bf16)
            # accumulator [128, NB, 65]: cols 0:64 = Y, col 64 = S
            acc = pin.tile([P, NB, D + 1], f32)
            nc.vector.memset(acc[:], 0.0)
            nc.vector.memset(xnp[:, NB - 1, :], 0.0)
            qrow = pin.tile([P, NB], f32)
            invr = pin.tile([P, NB], f32)
            nc.vector.memset(qrow[:, NB - 1:NB], 0.0)
            nc.vector.memset(invr[:, NB - 1:NB], 0.0)

            # --- normalize the row shard into xnp / xnpb -------------------
            with tc.tile_pool(name="dense", bufs=2) as dense:
                for r0, r1, np_ in [(0, FULLB, P), (FULLB, SH, RUNT)]:
                    nblk = (r1 - r0) // np_
                    xin = dense.tile([P, NB, D], f32, tag="xin")
                    src = xs_ext[r0:r1, :].rearrange("(b p) d -> p b d", p=np_)
                    bsl = slice(r0 // P, r0 // P + nblk)
                    nc.sync.dma_start(out=xin[:np_, bsl, :], in_=src)
                    sq = dense.tile([P, NB, D], f32, tag="sq")
                    nc.vector.tensor_tensor(out=sq[:np_, bsl, :],
                                            in0=xin[:np_, bsl, :],
                                            in1=xin[:np_, bsl, :], op=OP.mult)
                    nc.vector.tensor_reduce(qrow[:np_, bsl], sq[:np_, bsl, :],
                                            axis=AX.X, op=OP.add)
                    lnq = dense.tile([P, NB], f32, tag="lnq")
                    nc.scalar.activation(lnq[:np_, bsl], qrow[:np_, bsl],
                                         AF.Ln)
                    nc.scalar.activation(invr[:np_, bsl], lnq[:np_, bsl],
                                         AF.Exp, scale=-0.5)
                    nc.vector.tensor_tensor(
                        out=xnp[:np_, bsl, :], in0=xin[:np_, bsl, :],
                        in1=invr[:np_, bsl].broadcast_to([np_, nblk, D]),
                        op=OP.mult)
                nc.scalar.activation(xnpb[:], xnp[:], AF.Identity)

            # --- edge tiles -------------------------------------------------
            import contextlib
            _es = contextlib.ExitStack()
            big = _es.enter_context(tc.tile_pool(name="big", bufs=3))
            mid = _es.enter_context(tc.tile_pool(name="mid", bufs=3))
            epi = _es.enter_context(tc.tile_pool(name="epi", bufs=2))

            tiles_left = [tiles_per_hb[0][b] + tiles_per_hb[1][b]
                          for b in range(NB)]
            pending = [b for b in range(NB) if tiles_left[b] == 0]

            def emit_epilogue(blo, bhi):
                # out[r] = (Y + s_self*x_r) / (S + s_self) for blocks blo..bhi
                nb_ = bhi - blo + 1
                bs = slice(blo, bhi + 1)
                t1 = epi.tile([P, 8], f32, tag="t1")
                nc.vector.tensor_tensor(out=t1[:, :nb_], in0=qrow[:, bs],
                                        in1=invr[:, bs], op=OP.mult)
                xsc = epi.tile([P, 8], f32, tag="xsc")
                nc.vector.tensor_copy(out=xsc[:, :nb_], in_=t1[:, :nb_])
                nc.vector.tensor_tensor(out=t1[:, :nb_], in0=t1[:, :nb_],
                                        in1=invr[:, bs], op=OP.mult)
                ss = epi.tile([P, 8], f32, tag="ss")
                nc.scalar.activation(ss[:, :nb_], t1[:, :nb_], AF.Exp,
                                     scale=beta_val)
                St = epi.tile([P, 8, 1], f32, tag="St")
                nc.vector.tensor_tensor(
                    out=St[:, :nb_, :], in0=acc[:, bs, D:D + 1],
                    in1=ss[:, :nb_].unsqueeze(2), op=OP.add)
                rec = epi.tile([P, 8, 1], f32, tag="rec")
                nc.vector.reciprocal(rec[:, :nb_, :], St[:, :nb_, :])
                # x_r = xn_r * (q*inv); scale by s_self
                nc.vector.tensor_tensor(out=xsc[:, :nb_], in0=xsc[:, :nb_],
                                        in1=ss[:, :nb_], op=OP.mult)
                yt = epi.tile([P, 8, D], f32, tag="yt")
                nc.vector.tensor_tensor(
                    out=yt[:, :nb_, :], in0=xnp[:, bs, :],
                    in1=xsc[:, :nb_].unsqueeze(2).broadcast_to([P, nb_, D]),
                    op=OP.mult)
                nc.vector.tensor_tensor(out=yt[:, :nb_, :], in0=yt[:, :nb_, :],
                                        in1=acc[:, bs, 0:D], op=OP.add)
                nc.vector.tensor_tensor(
                    out=yt[:, :nb_, :], in0=yt[:, :nb_, :],
                    in1=rec[:, :nb_, :].broadcast_to([P, nb_, D]),
                    op=OP.mult)
                nfull = nb_ - 1 if bhi == NB - 1 else nb_
                if nfull > 0:
                    nc.sync.dma_start(
                        out=out_ext[blo * P:(blo + nfull) * P, :].rearrange(
                            "(b p) d -> p b d", p=P),
                        in_=yt[:, 0:nfull, :])
                if bhi == NB - 1:
                    nc.sync.dma_start(
                        out=out_ext[FULLB:SH, :].rearrange(
                            "(b p) d -> p b d", p=RUNT),
                        in_=yt[:RUNT, nb_ - 1:nb_, :])

            def flush_pending(force=False):
                while pending and (len(pending) >= 8 or force):
                    grp = sorted(pending)[:8]
                    # contiguous prefix
                    run = [grp[0]]
                    for b in grp[1:]:
                        if b == run[-1] + 1 and len(run) < 8:
                            run.append(b)
                        else:
                            break
                    for b in run:
                        pending.remove(b)
                    emit_epilogue(run[0], run[-1])

            t_global = 0
            emit_xhi_copy()
            for ci in range(n_chunks):
                nt = min(CTILE, n_tiles - ci * CTILE)
                gci = io.tile([P, C // 16], i16)
                nc.sync.dma_start(out=gci[:], in_=gc_ext[ci, :, :])
                rloc = io.tile([P, CTILE], bf16)
                nc.sync.dma_start(out=rloc[:], in_=rl_ext[ci, :, :])
                mki = io.tile([P, CTILE], f32)
                nc.scalar.dma_start(out=mki[:], in_=mk_ext[ci, :, :])
                # transposed row-ids, replicated to all partitions by a
                # stride-0-source DMA
                rltR = big.tile([P, C], bf16, tag="rltR")
                nc.scalar.dma_start(
                    out=rltR[:],
                    in_=rt_ext[ci:ci + 1, :].broadcast_to([P, C]))

                # gather x[col] for the whole chunk (1 desc/edge on the Q7)
                Gc = big.tile([P, CTILE, D], f32, tag="Gc")
                h0 = tile_half[ci * CTILE]
                ctab = x_ext[0:HALF, :] if h0 == 0 else xhi_t[:, :]
                nc.gpsimd.dma_gather(Gc[:], ctab, gci[:], C, C, D,
                                     single_packet=False)

                # GcS = [x_c | 1]  (SBUF->SBUF DMA for the strided copy)
                GcS = big.tile([P, CTILE, D + 1], f32, tag="GcS")
                nc.sync.dma_start(out=GcS[:, :, 0:D], in_=Gc[:])
                nc.vector.memset(GcS[:, :, D:D + 1], 1.0)

                # qc = ||x_c||^2 per edge
                sqc = big.tile([P, CTILE, D], f32, tag="sqc")
                nc.vector.tensor_tensor(out=sqc[:], in0=Gc[:], in1=Gc[:],
                                        op=OP.mult)
                qc = mid.tile([P, CTILE], f32, tag="qc")
                nc.vector.tensor_reduce(qc[:], sqc[:], axis=AX.X, op=OP.add)
                lnc = mid.tile([P, CTILE], f32, tag="lnc")
                nc.scalar.activation(lnc[:], qc[:], AF.Ln)
                ic = mid.tile([P, CTILE], f32, tag="ic")
                nc.scalar.activation(ic[:], lnc[:], AF.Exp, scale=-0.5)

                # walk the chunk's tiles by same-block runs
                ti = 0
                while ti < nt:
                    b = tile_blocks[t_global + ti]
                    run = 1
                    while (ti + run < nt
                           and tile_blocks[t_global + ti + run] == b):
                        run += 1
                    red = psacc.tile([P, D + 1], f32, tag="red", space="PSUM")
                    gdone = 0
                    while gdone < run:
                        g = min(4, run - gdone)
                        t0 = ti + gdone
                        tsl = slice(t0, t0 + g)
                        # EQ[p_e, (tt, i)] = (i == rloc[p_e, tt])
                        eq = mid.tile([P, 4, P], bf16, tag="eq")
                        nc.vector.tensor_tensor(
                            out=eq[:, :g, :],
                            in0=iotf[:].unsqueeze(1).broadcast_to([P, g, P]),
                            in1=rloc[:, tsl].unsqueeze(2).broadcast_to([P, g, P]),
                            op=OP.is_equal)
                        # EQT[p_r, (tt, e)] = (rlt[(tt,e)] == p_r)
                        eqt = mid.tile([P, 4, P], bf16, tag="eqt")
                        nc.vector.tensor_tensor(
                            out=eqt[:, :g, :],
                            in0=rltR[:, t0 * P:(t0 + g) * P].rearrange(
                                "p (t e) -> p t e", t=g),
                            in1=iop[:].unsqueeze(2).broadcast_to([P, g, P]),
                            op=OP.is_equal)
                        # expansion: xn_r per edge
                        gre = ps.tile([P, 4, D], f32, tag="gre", space="PSUM")
                        for k in range(g):
                            nc.tensor.matmul(out=gre[:, k, :],
                                             lhsT=eqt[:, k, :],
                                             rhs=xnpb[:, b, :],
                                             start=True, stop=True)
                        # d = <xn_r, x_c>, s = exp(beta*d*inv_c)*mask
                        prd = mid.tile([P, 4, D], f32, tag="prd")
                        nc.vector.tensor_tensor(out=prd[:, :g, :],
                                                in0=gre[:, :g, :],
                                                in1=Gc[:, tsl, :], op=OP.mult)
                        dd = mid.tile([P, 4], f32, tag="dd")
                        nc.vector.tensor_reduce(dd[:, :g], prd[:, :g, :],
                                                axis=AX.X, op=OP.add)
                        sv = mid.tile([P, 4], f32, tag="sv")
                        nc.vector.tensor_tensor(out=sv[:, :g], in0=dd[:, :g],
                                                in1=ic[:, tsl], op=OP.mult)
                        nc.scalar.activation(sv[:, :g], sv[:, :g], AF.Exp,
                                             scale=beta_val)
                        nc.vector.tensor_tensor(out=sv[:, :g], in0=sv[:, :g],
                                                in1=mki[:, tsl], op=OP.mult)
                        # rhs = [s * x_c | s] in bf16
                        rhs = mid.tile([P, 4, D + 1], bf16, tag="rhs")
                        nc.vector.tensor_tensor(
                            out=rhs[:, :g, :], in0=GcS[:, tsl, :],
                            in1=sv[:, :g].unsqueeze(2).broadcast_to([P, g, D + 1]),
                            op=OP.mult)
                        # reduction accumulate into the run's psum
                        for k in range(g):
                            nc.tensor.matmul(out=red[:], lhsT=eq[:, k, :],
                                             rhs=rhs[:, k, :],
                                             start=(gdone + k == 0),
                                             stop=(gdone + k == run - 1))
                        gdone += g
                    nc.vector.tensor_tensor(out=acc[:, b, :], in0=acc[:, b, :],
                                            in1=red[:], op=OP.add)
                    tiles_left[b] -= run
                    if tiles_left[b] == 0:
                        pending.append(b)
                    ti += run
                flush_pending()
                t_global += nt
            flush_pending(force=True)
            _es.close()

    nc.finalize()
    return nc, n_chunks, C


def _prepare(x, edge_index, beta):
    row = np.asarray(edge_index[0], dtype=np.int64)
    col = np.asarray(edge_index[1], dtype=np.int64)
    owner = row // SH
    half = (col >= HALF).astype(np.int64)
    rel_row = row - owner * SH
    rel_col = col - half * HALF

    counts = np.zeros((CORES, 2, NB), dtype=np.int64)
    sel = {}
    for k in range(CORES):
        for h in range(2):
            m = (owner == k) & (half == h)
            rr = rel_row[m]
            rc = rel_col[m]
            o = np.argsort(rr, kind="stable")
            rr = rr[o]
            rc = rc[o]
            blk = rr // P
            counts[k, h] = np.bincount(blk, minlength=NB)
            sel[(k, h)] = (rr, rc, blk)

    tiles_per_hb = [[0] * NB for _ in range(2)]
    for h in range(2):
        for b in range(NB):
            mx = int(counts[:, h, b].max())
            tiles_per_hb[h][b] = (mx + P - 1) // P
    for h in range(2):
        extra = (-sum(tiles_per_hb[h])) % CTILE
        tiles_per_hb[h][NB - 1] += extra
    return tiles_per_hb, sel


def _make_inputs(tiles_per_hb, sel, x_np, n_chunks, C):
    bfnp = _bf16np()
    n_tiles = sum(tiles_per_hb[0]) + sum(tiles_per_hb[1])
    total = n_tiles * P
    in_maps = []
    for k in range(CORES):
        gcol = np.zeros(total, dtype=np.int16)
        rloc = np.zeros(total, dtype=np.float32)
        mask = np.zeros(total, dtype=np.float32)
        pos = 0
        for h in range(2):
            rr, rc, blk = sel[(k, h)]
            border = range(NB)
            for b in border:
                cap = tiles_per_hb[h][b] * P
                m = blk == b
                nb_ = int(m.sum())
                if nb_:
                    gcol[pos:pos + nb_] = rc[m]
                    rloc[pos:pos + nb_] = (rr[m] - b * P).astype(np.float32)
                    mask[pos:pos + nb_] = 1.0
                if cap > nb_:
                    rloc[pos + nb_:pos + cap] = 127.0
                pos += cap
        gc = np.zeros((n_chunks, P, C // 16), dtype=np.int16)
        rl = np.zeros((n_chunks, P, CTILE), dtype=bfnp)
        rt = np.zeros((n_chunks, C), dtype=bfnp)
        mk = np.zeros((n_chunks, P, CTILE), dtype=np.float32)
        for ci in range(n_chunks):
            sl = slice(ci * C, (ci + 1) * C)
            gc[ci] = _wrap16(gcol[sl])
            rl[ci] = rloc[sl].reshape(CTILE, P).T.astype(bfnp)
            rt[ci] = rloc[sl].astype(bfnp)
            mk[ci] = mask[sl].reshape(CTILE, P).T
        in_maps.append({
            "x": x_np,
            "xshard": np.ascontiguousarray(x_np[k * SH:(k + 1) * SH]),
            "gc": gc,
            "rloc": rl,
            "rlt": rt,
            "mk": mk,
        })
    return in_maps


def kernel(x, edge_index, beta, _trace=False):
    from concourse.bass_utils import run_bass_kernel_spmd

    beta_val = float(np.asarray(beta).reshape(-1)[0])
    x_np = np.ascontiguousarray(np.asarray(x, dtype=np.float32))
    tiles_per_hb, sel = _prepare(x, edge_index, beta)

    key = (tuple(tiles_per_hb[0]), tuple(tiles_per_hb[1]), round(beta_val, 9))
    if key not in _graph_cache:
        _graph_cache[key] = _build(tiles_per_hb, beta_val)
    nc, n_chunks, C = _graph_cache[key]

    in_maps = _make_inputs(tiles_per_hb, sel, x_np, n_chunks, C)
    res = run_bass_kernel_spmd(nc, in_maps, core_ids=list(range(CORES)),
                               trace=_trace)
    out = np.concatenate([res.results[k]["out"] for k in range(CORES)], axis=0)
    kernel.last_exec_time_ns = res.exec_time_ns
    kernel.last_results = res.results
    return out.astype(np.float32)


kernel.last_exec_time_ns = None
